# revision 9
# baseline (speedup 1.0000x reference)
"""GIN message-passing (graph-masked autoencoder step) on 8 Trainium2 NeuronCores.

Strategy (node-sharded, feature-major):
  - 50000 nodes split 8 ways (6250/core, padded to 6272 = 49x128-row windows).
    Full feature table replicated per core in DRAM (fp16 for gathers); each
    core owns its node-slice.
  - segment_sum: edges bucketed by dst core/window on host, gathered in bulk
    via dma_gather (int16 idx -> table split in two views), reduced on the
    TensorEngine as X_tile.T @ onehot accumulating into PSUM (transposed
    aggregate, feature-major). Onehot tiles are built on-device with a DVE
    is_equal against an iota constant from 1-float-per-edge slot vectors.
    The GIN self-term is one identity-onehot tile per window fed by a
    contiguous DMA from the core's own slice.
  - GEMMs with pre-transposed weights keep activations [feature x rows], so
    BatchNorm stats are bn_stats/bn_aggr along the free axis, globalized with
    a 2KB AllReduce; normalize+ReLU is one fused ScalarE activation.
  - Per layer: gather+segsum+GEMM1+stats -> AllReduce -> norm+GEMM2+stats ->
    AllReduce -> norm+transpose+write slice -> AllGather (layers 1,2).
  - The tiny 273-node target encoder and the final cosine loss run on host.
"""
import os
import numpy as np
from contextlib import ExitStack

import concourse.bass as bass
import concourse.bacc as bacc
import concourse.tile as tile
import concourse.mybir as mybir
from concourse.bass_utils import run_bass_kernel_spmd
from concourse import library_config

M = 8          # cores
D = 256        # feature dim
W = 128        # window rows
L = 3          # layers
F32 = mybir.dt.float32
I16 = mybir.dt.int16

# gather dtype (validated: fp16 gathers give ~5e-6 final rel err)
DT = mybir.dt.float16
DT_NP = np.float16
GATHER_GROUP = 3  # windows per dma_gather call

LAST_EXEC_NS = None
LAST_PROFILE = None


# --------------------------------------------------------------------------
# host-side graph structure
# --------------------------------------------------------------------------
class Structure:
    pass


def build_structure(src, dst, n_nodes, npc, split):
    assert n_nodes == M * npc
    rpc = ((npc + W - 1) // W) * W
    wpc = rpc // W
    s = Structure()
    s.n_nodes, s.npc, s.rpc, s.wpc, s.split = n_nodes, npc, rpc, wpc, split
    s.npad = M * rpc

    src = np.asarray(src, np.int64)
    dst = np.asarray(dst, np.int64)
    c = dst // npc
    ld = dst % npc
    w = ld // W
    slot = ld % W
    srcrow = rpc * (src // npc) + src % npc
    half = (srcrow >= split).astype(np.int64)
    idxval = srcrow - split * half
    assert split <= 32768 and (s.npad - split) <= 32768
    assert idxval.max(initial=0) < 32768

    key = (c * wpc + w) * 2 + half
    counts = np.bincount(key, minlength=M * wpc * 2).reshape(M, wpc, 2)
    maxcnt = counts.max(axis=0)
    T = -(-maxcnt // W)
    s.T_lo = T[:, 0].copy()
    s.T_hi = T[:, 1].copy()
    s.tiles_w = s.T_lo + s.T_hi + 1
    s.tile_off = np.concatenate([[0], np.cumsum(s.tiles_w)]).astype(np.int64)
    s.tiles_tot = int(s.tile_off[-1])
    s.lo_off = np.concatenate([[0], np.cumsum(s.T_lo * W)]).astype(np.int64)
    s.hi_off = np.concatenate([[0], np.cumsum(s.T_hi * W)]).astype(np.int64)
    s.n_lo = int(s.lo_off[-1])
    s.n_hi = int(s.hi_off[-1])

    order = np.argsort(key, kind="stable")
    ranks = np.empty_like(order)
    sec_start = np.concatenate([[0], np.cumsum(counts.reshape(-1))])
    ranks[order] = np.arange(len(order)) - np.repeat(sec_start[:-1], counts.reshape(-1))

    s.idx_lo = np.zeros((M, max(s.n_lo, 16)), np.int16)
    s.idx_hi = np.zeros((M, max(s.n_hi, 16)), np.int16)
    s.dvec = np.full((M, W, s.tiles_tot), 255.0, np.float32)
    selfcol = s.tile_off[:-1] + s.T_lo + s.T_hi
    s.dvec[:, :, selfcol] = np.arange(W, dtype=np.float32)[None, :, None]

    for name, hsel, idxarr, off, tbase in (
        ("lo", half == 0, s.idx_lo, s.lo_off, s.tile_off[:-1]),
        ("hi", half == 1, s.idx_hi, s.hi_off, s.tile_off[:-1] + s.T_lo),
    ):
        e = np.flatnonzero(hsel)
        idxarr[c[e], off[w[e]] + ranks[e]] = idxval[e].astype(np.int16)
        s.dvec[c[e], ranks[e] % W, tbase[w[e]] + ranks[e] // W] = slot[e]
    return s


def idx_sbuf_layout(flat):
    n = flat.shape[-1]
    assert n % 16 == 0
    a = flat.reshape(n // 16, 16).T
    return np.ascontiguousarray(np.tile(a, (8, 1)))


def pad_table(h, npc, rpc):
    n, d = h.shape
    out = np.zeros((M, rpc, d), h.dtype)
    out[:, :npc] = h.reshape(M, npc, d)
    return out.reshape(M * rpc, d)


# --------------------------------------------------------------------------
# bass program
# --------------------------------------------------------------------------
def build_program(s):
    npc, rpc, wpc, split, npad = s.npc, s.rpc, s.wpc, s.split, s.npad
    n_lo_c = max(s.n_lo, 16) // 16
    n_hi_c = max(s.n_hi, 16) // 16
    maxT = int(s.tiles_w.max())

    # window groups for gather calls
    groups = [list(range(g, min(g + GATHER_GROUP, wpc)))
              for g in range(0, wpc, GATHER_GROUP)]
    glo = [int(s.lo_off[g[-1] + 1] - s.lo_off[g[0]]) for g in groups]
    ghi = [int(s.hi_off[g[-1] + 1] - s.hi_off[g[0]]) for g in groups]
    max_glo = max(glo) // W if s.n_lo else 0
    max_ghi = max(ghi) // W if s.n_hi else 0

    nc = bacc.Bacc("TRN2", target_bir_lowering=False, debug=False, num_devices=M)

    h0_full = nc.dram_tensor("h0_full", [npad, D], DT, kind="ExternalInput")
    h0_slice = nc.dram_tensor("h0_slice", [rpc, D], DT, kind="ExternalInput")
    idx_lo_d = nc.dram_tensor("idx_lo", [128, n_lo_c], I16, kind="ExternalInput")
    idx_hi_d = nc.dram_tensor("idx_hi", [128, n_hi_c], I16, kind="ExternalInput")
    dvec_d = nc.dram_tensor("dvec", [W, s.tiles_tot], DT, kind="ExternalInput")
    iota_d = nc.dram_tensor("iota", [128, 128], DT, kind="ExternalInput")
    ident_d = nc.dram_tensor("ident", [128, 128], F32, kind="ExternalInput")
    w1t_d = nc.dram_tensor("w1t", [L, 2, 2, 128, 128], F32, kind="ExternalInput")
    w2t_d = nc.dram_tensor("w2t", [L, 2, 2, 128, 128], F32, kind="ExternalInput")
    gb_d = nc.dram_tensor("gb", [L, 2, 2, 2, 128], F32, kind="ExternalInput")
    h3_d = nc.dram_tensor("h3", [rpc, D], F32, kind="ExternalOutput")
    debug = bool(int(os.environ.get("KERNEL_DEBUG_TAPS", "0")))
    if debug:
        dbg_agg = nc.dram_tensor("dbg_agg", [128, 2, rpc], F32, kind="ExternalOutput")
        dbg_t = nc.dram_tensor("dbg_t", [2, 128, rpc], F32, kind="ExternalOutput")
        dbg_m = nc.dram_tensor("dbg_m", [2, 128, rpc], F32, kind="ExternalOutput")
        dbg_kc = nc.dram_tensor("dbg_kc", [2, 128, 4], F32, kind="ExternalOutput")

    rg = [list(range(M))]

    def wcnt(w):  # real rows in window
        return max(0, min(W, npc - w * W))

    with tile.TileContext(nc) as tc, ExitStack() as ctx:
        nc.gpsimd.load_library(library_config.mlp)
        singles = ctx.enter_context(tc.tile_pool(name="singles", bufs=1))
        gpool = ctx.enter_context(tc.tile_pool(name="gather", bufs=2))
        spool = ctx.enter_context(tc.tile_pool(name="selfp", bufs=3))
        opool = ctx.enter_context(tc.tile_pool(name="oh", bufs=3))
        evac = ctx.enter_context(tc.tile_pool(name="evac", bufs=3))
        hout = ctx.enter_context(tc.tile_pool(name="hout", bufs=3))
        stp = ctx.enter_context(tc.tile_pool(name="stats", bufs=3))
        wst = ctx.enter_context(tc.tile_pool(name="winstats", bufs=2))
        pagg_p = ctx.enter_context(tc.tile_pool(name="pagg", bufs=2, space="PSUM"))
        pgem_p = ctx.enter_context(tc.tile_pool(name="pgem", bufs=2, space="PSUM"))
        ptr_p = ctx.enter_context(tc.tile_pool(name="ptr", bufs=2, space="PSUM"))
        dram = ctx.enter_context(tc.tile_pool(name="dram", bufs=2, space="DRAM"))
        dram1 = ctx.enter_context(tc.tile_pool(name="dram1", bufs=2, space="DRAM"))

        # persistent SBUF state
        idxlo_sb = singles.tile([128, n_lo_c], I16)
        idxhi_sb = singles.tile([128, n_hi_c], I16)
        dvec_sb = singles.tile([W, s.tiles_tot], DT)
        iota_sb = singles.tile([128, 128], DT)
        ident_sb = singles.tile([128, 128], F32)
        w1t_sb = singles.tile([128, L * 4, 128], F32)
        w2t_sb = singles.tile([128, L * 4, 128], F32)
        gb_sb = singles.tile([128, L * 8], F32)
        eps_sb = singles.tile([128, 1], F32)
        actT = [singles.tile([128, rpc], F32, tag=f"actT{c}", name=f"actT{c}") for c in range(2)]

        nc.sync.dma_start(idxlo_sb[:], idx_lo_d[:])
        nc.sync.dma_start(idxhi_sb[:], idx_hi_d[:])
        nc.sync.dma_start(dvec_sb[:], dvec_d[:])
        nc.sync.dma_start(iota_sb[:], iota_d[:])
        nc.sync.dma_start(ident_sb[:], ident_d[:])
        nc.sync.dma_start(w1t_sb[:], w1t_d.ap().rearrange("l i o p f -> p (l i o) f"))
        nc.sync.dma_start(w2t_sb[:], w2t_d.ap().rearrange("l i o p f -> p (l i o) f"))
        nc.sync.dma_start(gb_sb[:], gb_d.ap().rearrange("l b c g p -> p (l b c g)"))
        nc.vector.memset(eps_sb[:], 1e-5)

        hfull_t = [None, None]
        slice_t = [None, None]
        for l in range(2):
            hfull_t[l] = dram1.tile([npad, D], DT, tag="hfull", name=f"hfull{l}", addr_space="Shared")
            slice_t[l] = dram1.tile([rpc, D], DT, tag="slice", name=f"slice{l}")

        def bn_apply_coeffs(l, bn, st):
            """AllReduce exact [Sx, Sxx]; return kc tile [128,4] = [k0,k1,c0,c1].

            bn_stats rows are (cnt_e, mean_e, cnt*var_e, cnt_o, mean_o, cnt*var_o)
            per window; combine exactly: Sx = sum cnt*mean, Sxx = sum
            (cnt*var + cnt*mean^2)."""
            pack = stp.tile([128, 4], F32, tag="pack")
            for c in range(2):
                a = wst.tile([128, wpc], F32, tag="bna")
                b = wst.tile([128, wpc], F32, tag="bnb")
                sxx = wst.tile([128, wpc], F32, tag="bnsxx")
                t1 = wst.tile([128, wpc], F32, tag="bnt1")
                nc.vector.tensor_mul(a[:], st[c][:, :, 0], st[c][:, :, 1])
                nc.vector.tensor_mul(b[:], st[c][:, :, 3], st[c][:, :, 4])
                nc.vector.tensor_add(sxx[:], st[c][:, :, 2], st[c][:, :, 5])
                nc.vector.tensor_mul(t1[:], a[:], st[c][:, :, 1])
                nc.vector.tensor_add(sxx[:], sxx[:], t1[:])
                nc.vector.tensor_mul(t1[:], b[:], st[c][:, :, 4])
                nc.vector.tensor_add(sxx[:], sxx[:], t1[:])
                nc.vector.tensor_add(a[:], a[:], b[:])
                nc.vector.reduce_sum(pack[:, 2 * c: 2 * c + 1], a[:],
                                     axis=mybir.AxisListType.X)
                nc.vector.reduce_sum(pack[:, 2 * c + 1: 2 * c + 2], sxx[:],
                                     axis=mybir.AxisListType.X)
            arin = dram.tile([128, 4], F32, tag="arin")
            arout = dram.tile([128, 4], F32, tag="arout", addr_space="Shared")
            nc.sync.dma_start(arin[:], pack[:])
            nc.gpsimd.collective_compute(
                "AllReduce", mybir.AluOpType.add, replica_groups=rg,
                ins=[arin.opt()], outs=[arout.opt()])
            ar = stp.tile([128, 4], F32, tag="ar")
            nc.sync.dma_start(ar[:], arout[:])
            kc = stp.tile([128, 4], F32, tag="kc")
            mg = stp.tile([128, 2], F32, tag="mg")
            inv_n = 1.0 / s.n_nodes
            for c in range(2):
                # global mean / E[x^2]
                nc.scalar.mul(mg[:, c: c + 1], ar[:, 2 * c: 2 * c + 1], inv_n)
                nc.scalar.mul(ar[:, 2 * c + 1: 2 * c + 2], ar[:, 2 * c + 1: 2 * c + 2], inv_n)
                v = stp.tile([128, 1], F32, tag="var")
                nc.vector.tensor_mul(v[:], mg[:, c: c + 1], mg[:, c: c + 1])
                nc.vector.tensor_tensor(out=v[:], in0=ar[:, 2 * c + 1: 2 * c + 2],
                                        in1=v[:], op=mybir.AluOpType.subtract)
                # sd = sqrt(var + eps); rinv = 1/sd
                nc.scalar.activation(out=v[:], in_=v[:],
                                     func=mybir.ActivationFunctionType.Sqrt,
                                     bias=eps_sb[:], scale=1.0)
                nc.vector.reciprocal(out=v[:], in_=v[:])
                g_ap = gb_sb[:, (((l * 2 + bn) * 2 + c) * 2 + 0): (((l * 2 + bn) * 2 + c) * 2 + 1)]
                b_ap = gb_sb[:, (((l * 2 + bn) * 2 + c) * 2 + 1): (((l * 2 + bn) * 2 + c) * 2 + 2)]
                nc.vector.tensor_mul(kc[:, c: c + 1], g_ap, v[:])
                nc.vector.tensor_mul(v[:], mg[:, c: c + 1], kc[:, c: c + 1])
                nc.vector.tensor_tensor(out=kc[:, 2 + c: 3 + c], in0=b_ap, in1=v[:],
                                        op=mybir.AluOpType.subtract)
            return kc

        for l in range(L):
            hsrc_full = h0_full.ap() if l == 0 else hfull_t[l - 1][:]
            hsrc_slice = h0_slice.ap() if l == 0 else slice_t[l - 1][:]
            st1 = [wst.tile([128, wpc, 6], F32, tag=f"st1{c}", name=f"st1_{c}") for c in range(2)]
            st2 = [wst.tile([128, wpc, 6], F32, tag=f"st2{c}", name=f"st2_{c}") for c in range(2)]

            # ---------------- phase A ----------------
            for gi, grp in enumerate(groups):
                xlo = xhi = None
                if glo[gi]:
                    xlo = gpool.tile([128, max_glo, D], DT, tag="xlo")
                    c0 = int(s.lo_off[grp[0]]) // 16
                    nc.gpsimd.dma_gather(
                        xlo[:, : glo[gi] // W, :], hsrc_full[0:split, :],
                        idxlo_sb[:, c0: c0 + glo[gi] // 16], glo[gi], glo[gi], D,
                        single_packet=False)
                if ghi[gi]:
                    xhi = gpool.tile([128, max_ghi, D], DT, tag="xhi")
                    c0 = int(s.hi_off[grp[0]]) // 16
                    nc.gpsimd.dma_gather(
                        xhi[:, : ghi[gi] // W, :], hsrc_full[split:npad, :],
                        idxhi_sb[:, c0: c0 + ghi[gi] // 16], ghi[gi], ghi[gi], D,
                        single_packet=False)
                for w in grp:
                    tw = int(s.tiles_w[w])
                    to = int(s.tile_off[w])
                    oh = opool.tile([128, maxT, 128], DT, tag="oh")
                    nc.vector.tensor_tensor(
                        out=oh[:, :tw, :],
                        in0=dvec_sb[:, to: to + tw].to_broadcast([W, tw, 128]),
                        in1=iota_sb[:].rearrange("p (t f) -> p t f", t=1).broadcast_to([128, tw, 128]),
                        op=mybir.AluOpType.is_equal)
                    xself = spool.tile([128, D], DT, tag="xself")
                    nc.sync.dma_start(xself[:], hsrc_slice[w * W:(w + 1) * W, :])
                    # segment-sum matmuls: aggT[i] = sum_t X_t[:, chunk i].T @ onehot_t
                    pagg = pagg_p.tile([128, 2, 128], F32, tag="pagg")
                    lo0 = (int(s.lo_off[w]) - int(s.lo_off[grp[0]])) // W
                    hi0 = (int(s.hi_off[w]) - int(s.hi_off[grp[0]])) // W
                    srcs = ([(xlo, lo0 + t, t) for t in range(int(s.T_lo[w]))]
                            + [(xhi, hi0 + t, int(s.T_lo[w]) + t) for t in range(int(s.T_hi[w]))]
                            + [(xself, None, tw - 1)])
                    for i in range(2):
                        for k, (buf, tloc, tcol) in enumerate(srcs):
                            lhsT = (buf[:, i * 128:(i + 1) * 128] if tloc is None
                                    else buf[:, tloc, i * 128:(i + 1) * 128])
                            nc.tensor.matmul(pagg[:, i, :], lhsT=lhsT, rhs=oh[:, tcol, :],
                                             start=(k == 0), stop=(k == len(srcs) - 1))
                    aggT = evac.tile([128, 2, 128], F32, tag="aggT")
                    nc.vector.tensor_copy(aggT[:], pagg[:])
                    if debug and l == 0:
                        nc.sync.dma_start(dbg_agg[:, :, w * W:(w + 1) * W], aggT[:])
                    # GEMM1: tT[o] = sum_i W1T[i,o].T @ aggT[i]
                    pt = pgem_p.tile([128, 2, 128], F32, tag="pgem")
                    for o in range(2):
                        for i in range(2):
                            nc.tensor.matmul(pt[:, o, :], lhsT=w1t_sb[:, l * 4 + i * 2 + o, :],
                                             rhs=aggT[:, i, :], start=(i == 0), stop=(i == 1))
                    for c in range(2):
                        nc.scalar.copy(actT[c][:, w * W:(w + 1) * W], pt[:, c, :])
                        nc.vector.bn_stats(out=st1[c][:, w, :],
                                           in_=actT[c][:, w * W: w * W + wcnt(w)])
                    if debug and l == 0:
                        for c in range(2):
                            nc.sync.dma_start(dbg_t[c, :, w * W:(w + 1) * W],
                                              actT[c][:, w * W:(w + 1) * W])

            kc1 = bn_apply_coeffs(l, 0, st1)
            if debug and l == 0:
                nc.sync.dma_start(dbg_kc[0], kc1[:])

            # ---------------- phase B ----------------
            for w in range(wpc):
                for c in range(2):
                    nc.scalar.activation(
                        out=actT[c][:, w * W:(w + 1) * W],
                        in_=actT[c][:, w * W:(w + 1) * W],
                        func=mybir.ActivationFunctionType.Relu,
                        bias=kc1[:, 2 + c: 3 + c], scale=kc1[:, c: c + 1])
                pm = pgem_p.tile([128, 2, 128], F32, tag="pgem")
                for o in range(2):
                    for i in range(2):
                        nc.tensor.matmul(pm[:, o, :], lhsT=w2t_sb[:, l * 4 + i * 2 + o, :],
                                         rhs=actT[i][:, w * W:(w + 1) * W],
                                         start=(i == 0), stop=(i == 1))
                for c in range(2):
                    nc.scalar.copy(actT[c][:, w * W:(w + 1) * W], pm[:, c, :])
                    nc.vector.bn_stats(out=st2[c][:, w, :],
                                       in_=actT[c][:, w * W: w * W + wcnt(w)])
                if debug and l == 0:
                    for c in range(2):
                        nc.sync.dma_start(dbg_m[c, :, w * W:(w + 1) * W],
                                          actT[c][:, w * W:(w + 1) * W])

            kc2 = bn_apply_coeffs(l, 1, st2)
            if debug and l == 0:
                nc.sync.dma_start(dbg_kc[1], kc2[:])

            # ---------------- phase C ----------------
            for w in range(wpc):
                for c in range(2):
                    nc.scalar.activation(
                        out=actT[c][:, w * W:(w + 1) * W],
                        in_=actT[c][:, w * W:(w + 1) * W],
                        func=mybir.ActivationFunctionType.Relu,
                        bias=kc2[:, 2 + c: 3 + c], scale=kc2[:, c: c + 1])
                ptr = ptr_p.tile([128, 2, 128], F32, tag="ptr")
                for c in range(2):
                    nc.tensor.transpose(ptr[:, c, :], actT[c][:, w * W:(w + 1) * W],
                                        ident_sb[:])
                hrow = hout.tile([128, 2, 128], F32, tag="hrow")
                nc.vector.tensor_copy(hrow[:], ptr[:])
                if l < L - 1:
                    hdt = hout.tile([128, D], DT, tag="hdt")
                    nc.vector.tensor_copy(hdt[:], hrow[:].rearrange("p a b -> p (a b)"))
                    nc.sync.dma_start(slice_t[l][w * W:(w + 1) * W, :], hdt[:])
                else:
                    nc.sync.dma_start(h3_d[w * W:(w + 1) * W, :],
                                      hrow[:].rearrange("p a b -> p (a b)"))
            if l < L - 1:
                nc.gpsimd.collective_compute(
                    "AllGather", mybir.AluOpType.bypass, replica_groups=rg,
                    ins=[slice_t[l].opt()], outs=[hfull_t[l].opt()])

    nc.compile()
    return nc


# --------------------------------------------------------------------------
# host-side helpers (small encoder, loss)
# --------------------------------------------------------------------------
def _np_bn(x, g, b):
    mu = x.mean(0)
    var = ((x - mu) ** 2).mean(0)
    return (x - mu) * (1.0 / np.sqrt(var + 1e-5)) * g + b


def _np_encoder(h, src, dst, W1, W2, g1, b1, g2, b2):
    h = h.astype(np.float32)
    for l in range(W1.shape[0]):
        acc = np.zeros_like(h)
        np.add.at(acc, dst, h[src])
        agg = h + acc
        mm = np.maximum(_np_bn(agg @ W1[l].T, g1[l], b1[l]), 0)
        mm = mm @ W2[l].T
        h = np.maximum(_np_bn(mm, g2[l], b2[l]), 0)
    return h


_CACHE = {}


def _get_program(s):
    key = (s.n_nodes, s.npc, s.split, tuple(s.T_lo), tuple(s.T_hi))
    if key not in _CACHE:
        _CACHE[key] = build_program(s)
    return _CACHE[key]


def run_encoder_device(s, rem, weights):
    """rem [n_nodes, D] f32; weights dict with W1,W2,g1,b1,g2,b2 [L,...].
    Returns h_final [n_nodes, D] f32."""
    global LAST_EXEC_NS, LAST_PROFILE
    npc, rpc = s.npc, s.rpc
    nc = _get_program(s)

    h0p = pad_table(rem.astype(np.float32), npc, rpc).astype(DT_NP)
    W1, W2 = weights["W1"], weights["W2"]
    w1t = np.zeros((L, 2, 2, 128, 128), np.float32)
    w2t = np.zeros((L, 2, 2, 128, 128), np.float32)
    for l in range(L):
        for i in range(2):
            for o in range(2):
                w1t[l, i, o] = W1[l][o * 128:(o + 1) * 128, i * 128:(i + 1) * 128].T
                w2t[l, i, o] = W2[l][o * 128:(o + 1) * 128, i * 128:(i + 1) * 128].T
    gb = np.zeros((L, 2, 2, 2, 128), np.float32)
    for l in range(L):
        for c in range(2):
            gb[l, 0, c, 0] = weights["g1"][l][c * 128:(c + 1) * 128]
            gb[l, 0, c, 1] = weights["b1"][l][c * 128:(c + 1) * 128]
            gb[l, 1, c, 0] = weights["g2"][l][c * 128:(c + 1) * 128]
            gb[l, 1, c, 1] = weights["b2"][l][c * 128:(c + 1) * 128]
    iota = np.broadcast_to(np.arange(128, dtype=DT_NP), (128, 128)).copy()
    ident = np.eye(128, dtype=np.float32)

    in_maps = []
    for c in range(M):
        in_maps.append({
            "h0_full": h0p,
            "h0_slice": np.ascontiguousarray(h0p[c * rpc:(c + 1) * rpc]),
            "idx_lo": idx_sbuf_layout(s.idx_lo[c]),
            "idx_hi": idx_sbuf_layout(s.idx_hi[c]),
            "dvec": s.dvec[c].astype(DT_NP),
            "iota": iota,
            "ident": ident,
            "w1t": w1t, "w2t": w2t, "gb": gb,
        })
    trace = bool(int(os.environ.get("KERNEL_TRACE", "0")))
    res = run_bass_kernel_spmd(nc, in_maps, core_ids=list(range(M)), trace=trace)
    LAST_EXEC_NS = res.exec_time_ns
    LAST_PROFILE = res.profile_json
    h = np.concatenate([res.results[c]["h3"][:npc] for c in range(M)], 0)
    return h


def kernel(feat, enc_mask_token, src, dst, ring_nodes, sub_src, sub_dst,
           on_W1, on_W2, on_g1, on_b1, on_g2, on_b2,
           tg_W1, tg_W2, tg_g1, tg_b1, tg_g2, tg_b2):
    feat = np.asarray(feat, np.float32)
    ring = np.asarray(ring_nodes, np.int64)
    rem = feat.copy()
    rem[ring] = np.asarray(enc_mask_token, np.float32)[0]

    n = feat.shape[0]
    s = build_structure(np.asarray(src), np.asarray(dst), n, n // M, 32768)
    h1 = run_encoder_device(s, rem, dict(W1=np.asarray(on_W1), W2=np.asarray(on_W2),
                                         g1=np.asarray(on_g1), b1=np.asarray(on_b1),
                                         g2=np.asarray(on_g2), b2=np.asarray(on_b2)))

    h2 = _np_encoder(feat[ring], np.asarray(sub_src, np.int64),
                     np.asarray(sub_dst, np.int64),
                     np.asarray(tg_W1), np.asarray(tg_W2), np.asarray(tg_g1),
                     np.asarray(tg_b1), np.asarray(tg_g2), np.asarray(tg_b2))

    x = h1[ring]
    xn = x / np.maximum(np.linalg.norm(x, axis=-1, keepdims=True), 1e-12)
    yn = h2 / np.maximum(np.linalg.norm(h2, axis=-1, keepdims=True), 1e-12)
    return np.float32((1.0 - (xn * yn).sum(-1)).mean())
